# revision 24
# baseline (speedup 1.0000x reference)
"""Trainium2 Bass kernel for nn_Attention_43628277793473.

Single-head attention, B=8, S=2048, H=1024:
  q = query @ Wq.T ; k = key @ Wk.T ; v = value @ Wv.T
  score = q @ k.T ; masked_fill(mask==0, -99999) ; softmax ; out = attn @ v

Sharding: data-parallel over the batch dim - one batch element per
NeuronCore (8 cores), no collectives.

Host-side prep (layout/dtype only, all dense FLOPs stay on device):
  - key compaction: masked keys contribute exactly zero (exp underflow),
    so only unmasked key/value rows are sent, padded to KT*128 (Spad),
    with a per-key bias column (0 real, -50000 padding).  Phase B only
    computes the first Spad2 = ceil(nmax/32)*32 key columns; the tail is
    zero-filled once.
  - pre-transposed inputs: XqT=[H,S] f32r, XkT=[H,Spad] fp16,
    XvT=[H,Spad] bf16, WvT=[H,H] bf16, Wq/Wk [H,H] fp16.  This removes
    every on-device PE transpose (the original kernel spent ~60us of PE
    time on transposes + their LDWEIGHTS) and halves the weight/key DMA.
    fp16 (not bf16) on the score path keeps rel-err at 6.2e-3; bf16
    there fails the 2e-2 gate (3e-2+).

Device algorithm per core (G-gram factorization - no q/k projections
are materialized, so no mid-kernel transposes are needed):
  A:  G[b,a]   = sum_o Wk[o,b] Wq[o,a]          (fp16, f32 psum)
  B:  C[a,j]   = sum_b G[b,a] XkT[b,j]          (fp16 -> f32r)
  C2: v[j,:]   = sum_h XvT[h,j] WvT[h,:]        (bf16)
  D1: scoreT   = C.T @ xqT ; expT = exp(+bias)  (f32r -> bf16)
  D2: ctx[i,:] = sum_j expT[j,i] v[j,:]         (bf16)
      Z[i]     = sum_j expT[j,i]  (stationary=expT slice, moving=ones,
                 lands [q,1] in PSUM - no transpose needed)
  out[i,:] = ctx[i,:] / Z[i]

Hardware facts this layout is built around (measured via NTFF traces):
  - matmul streams 1 col/cycle at 2.4GHz for f32r/fp16/bf16 alike;
    moving free dim is capped at 512 (walrus s3d3_mm_num_elements).
  - every matmul carries its own LDWEIGHTS (walrus pairs them 1:1;
    they cannot be deduped post-compile) which partially overlaps the
    previous matmul's stream.
  - a matmul PSUM output region must not start mid-bank or cross a 2KB
    PSUM bank boundary (corrupts accumulation -> NaN).
  - PE idle gaps >~1us drop the HAM clock to half for 5-25us, so every
    phase transition must be prefetched: weights stream in quad-group
    order during A, key tiles land during late A (queued behind the
    weights on the same DMA queue), xvt/wvT during B, xqT during C2.
  - queries run in two halves of 1024 so score PSUM (2x[128,512]),
    ctx PSUM (2x[128,1024]) and Z ([128,16]) coexist in the 8 PSUM
    banks with no pool churn inside phase D; D-phase pools are ordered
    so their banks land where no reader is pending.
"""
import os
from contextlib import ExitStack

import numpy as np
from ml_dtypes import bfloat16

_CACHE = {}

B, S, H, P = 8, 2048, 1024, 128


def _bchunks(spad):
    """Split [0, spad) into PSUM-bank-aligned column chunks of <=512
    f32 columns.  A matmul output region must not cross a 2KB PSUM
    bank boundary mid-instruction."""
    out = []
    j = 0
    while j < spad:
        w = min(512, spad - j)
        out.append((j, w))
        j += w
    return out


def _build(KT, Spad2):
    import concourse.tile as tile
    from concourse import bacc, mybir


    F32 = mybir.dt.float32
    F32R = mybir.dt.float32r
    F16 = mybir.dt.float16
    BF16 = mybir.dt.bfloat16
    EXP = mybir.ActivationFunctionType.Exp

    HT = H // P              # 8 hidden tiles
    Spad = KT * P            # compacted+padded key count
    assert Spad2 <= Spad
    NCH = H // 512           # 2 column chunks of the hidden dim
    QH = S // 2              # 1024 queries per half
    NBLK = S // P            # 16 query row-blocks

    nc = bacc.Bacc("TRN2", target_bir_lowering=False, debug=False,
                   num_devices=8)

    XqT = nc.dram_tensor("xqT", [H, S], F32R, kind="ExternalInput").ap()
    XkT = nc.dram_tensor("xkT", [H, Spad], F16, kind="ExternalInput").ap()
    XvT = nc.dram_tensor("xvT", [H, Spad], BF16, kind="ExternalInput").ap()
    WvT = nc.dram_tensor("wvT", [H, H], BF16, kind="ExternalInput").ap()
    Wq = nc.dram_tensor("Wq", [H, H], F16, kind="ExternalInput").ap()
    Wk = nc.dram_tensor("Wk", [H, H], F16, kind="ExternalInput").ap()
    Bias = nc.dram_tensor("bias", [P, KT], F32, kind="ExternalInput").ap()
    Out = nc.dram_tensor("out", [S, H], F32, kind="ExternalOutput").ap()

    with tile.TileContext(nc) as tc, ExitStack() as root:
        const = root.enter_context(tc.tile_pool(name="const", bufs=1))
        ones_f = const.tile([P, 1], F32, tag="onesf")
        nc.vector.memset(ones_f[:], 1.0)
        ones_b = const.tile([P, 1], BF16, tag="ones")
        nc.vector.tensor_copy(ones_b[:], ones_f[:])
        bias_cols = const.tile([P, KT], F32, tag="bias")
        nc.sync.dma_start(bias_cols[:], Bias[:])

        # long-lived score factor C[a,j] and C2 prefetch targets
        c_pool = root.enter_context(tc.tile_pool(name="c", bufs=1))
        c_tiles = [c_pool.tile([P, Spad], F32R, name=f"c{a}", tag=f"c{a}")
                   for a in range(HT)]
        if Spad2 < Spad:
            # phase B only fills key columns [0, Spad2); zero the tail so
            # the padded keys score 0 and exp(0 - 50000) underflows to 0.
            # (memset doesn't support f32r, so copy from an f32 zeros tile)
            zer = const.tile([P, Spad - Spad2], F32, tag="zer")
            nc.vector.memset(zer[:], 0.0)
            for a in range(HT):
                nc.vector.tensor_copy(c_tiles[a][:, Spad2:Spad], zer[:])
        xvt_pool = root.enter_context(tc.tile_pool(name="xvt", bufs=1))
        wvt_pool = root.enter_context(tc.tile_pool(name="wvt", bufs=1))
        xvt = [xvt_pool.tile([P, Spad], BF16, name=f"xvt{h}", tag=f"xvt{h}")
               for h in range(HT)]
        wvt = [wvt_pool.tile([P, H], BF16, name=f"wvt{h}", tag=f"wvt{h}")
               for h in range(HT)]
        v_pool = root.enter_context(tc.tile_pool(name="v", bufs=1))
        v_tiles = [v_pool.tile([P, H], BF16, name=f"v{j}", tag=f"v{j}")
                   for j in range(KT)]

        with ExitStack() as sP1:
            m_pool = sP1.enter_context(tc.tile_pool(name="mkq", bufs=1))
            xkt_pool = sP1.enter_context(tc.tile_pool(name="xkt", bufs=1))
            xkt = [xkt_pool.tile([P, Spad], F16, name=f"xkt{b}", tag=f"xkt{b}")
                   for b in range(HT)]
            m_tiles = [m_pool.tile([P, H], F16, name=f"m{b}", tag=f"m{b}")
                       for b in range(HT)]

            # phase A: G[b,a] = sum_o Wk[o,b] Wq[o,a].  Four quad-groups
            # (b-quad x wq-half), each filling 4 double-buffered [P,512]
            # PSUM tiles, so group n+1's matmuls never wait on group n's
            # PSUM copies.  DMA order matches consumption order: wk/wq
            # first halves, then wq second halves (group 1), then wk
            # second halves (groups 2-3), then the key tiles (phase B).
            with ExitStack() as sP2:
                w_pool = sP2.enter_context(tc.tile_pool(name="wkq", bufs=1))
                wq_pool = sP2.enter_context(tc.tile_pool(name="wqf", bufs=1))
                psA = sP2.enter_context(tc.tile_pool(name="psA", bufs=2, space="PSUM"))
                wk_t, wq_t = [], []
                for o in range(HT):
                    tk = w_pool.tile([P, H], F16, name=f"wk{o}", tag=f"wk{o}")
                    tq = wq_pool.tile([P, H], F16, name=f"wq{o}", tag=f"wq{o}")
                    if o == 0:
                        # split the critical first tiles so the very first
                        # matmul's operands land as early as possible
                        nc.scalar.dma_start(tk[:, 0:128], Wk[0:P, 0:128])
                        nc.gpsimd.dma_start(tq[:, 0:512], Wq[0:P, 0:512])
                        nc.scalar.dma_start(tk[:, 128:512], Wk[0:P, 128:512])
                    else:
                        nc.scalar.dma_start(tk[:, 0:512], Wk[o * P:(o + 1) * P, 0:512])
                        nc.gpsimd.dma_start(tq[:, 0:512], Wq[o * P:(o + 1) * P, 0:512])
                    wk_t.append(tk)
                    wq_t.append(tq)
                for o in range(HT):
                    nc.gpsimd.dma_start(
                        wq_t[o][:, 512:1024], Wq[o * P:(o + 1) * P, 512:1024])
                for o in range(HT):
                    nc.scalar.dma_start(
                        wk_t[o][:, 512:1024], Wk[o * P:(o + 1) * P, 512:1024])
                # xkt queued on gpsimd BEHIND the wq halves: the A-phase
                # weight DMAs get the full fabric first, the key tiles
                # stream during the rest of A (needed at phase B start)
                for b in range(HT):
                    nc.gpsimd.dma_start(xkt[b][:, 0:Spad2],
                                        XkT[b * P:(b + 1) * P, 0:Spad2])
                for wave in range(2):
                    for qch in range(NCH):
                        psQ = [psA.tile([P, 512], F32, name=f"g{wave}{qch}_{b}",
                                        tag=f"mmq{b}") for b in range(4)]
                        for o in range(HT):
                            for b in range(4):
                                babs = wave * 4 + b
                                nc.tensor.matmul(
                                    psQ[b][:],
                                    wk_t[o][:, babs * P:(babs + 1) * P],
                                    wq_t[o][:, qch * 512:(qch + 1) * 512],
                                    start=(o == 0), stop=(o == HT - 1))
                        for b in range(4):
                            dst = m_tiles[wave * 4 + b][:, qch * 512:(qch + 1) * 512]
                            if b % 2 == 0:
                                nc.vector.tensor_copy(dst, psQ[b][:])
                            else:
                                nc.scalar.copy(dst, psQ[b][:])

            # A's weight pools are now free; issue the C2 prefetch DMAs so
            # they land during phase B (root pools: no WAR wait).
            for h in range(HT):
                nc.gpsimd.dma_start(xvt[h][:], XvT[h * P:(h + 1) * P, :])
            for h in range(HT):
                nc.scalar.dma_start(wvt[h][:], WvT[h * P:(h + 1) * P, :])

            # phase B: C[a,j] = sum_b G[b,a] XkT[b,j], fp16 operands,
            # f32 PSUM.  psC is created before psB so phase C2's PSUM
            # banks are disjoint from B's (no WAR wait at B->C2).
            with ExitStack() as sPC:
                psC = sPC.enter_context(tc.tile_pool(name="psC", bufs=2, space="PSUM"))
                with ExitStack() as sP3:
                    psB = sP3.enter_context(
                        tc.tile_pool(name="psB", bufs=2, space="PSUM"))
                    for a in range(HT):
                        ps = psB.tile([P, Spad], F32, tag="mmB")
                        for b in range(HT):
                            for j0, w in _bchunks(Spad2):
                                nc.tensor.matmul(
                                    ps[:, j0:j0 + w],
                                    m_tiles[b][:, a * P:(a + 1) * P],
                                    xkt[b][:, j0:j0 + w],
                                    start=(b == 0), stop=(b == HT - 1))
                        if a % 2 == 0:
                            nc.vector.tensor_copy(
                                c_tiles[a][:, 0:Spad2], ps[:, 0:Spad2])
                        else:
                            nc.scalar.copy(
                                c_tiles[a][:, 0:Spad2], ps[:, 0:Spad2])
                # phase C2: v[j,:] = sum_h XvT[h,j] WvT[h,:]  (bf16)
                for kt in range(KT):
                    for ch in range(NCH):
                        ps = psC.tile([P, 512], F32, tag="mmC")
                        for h in range(HT):
                            nc.tensor.matmul(
                                ps[:], xvt[h][:, kt * P:(kt + 1) * P],
                                wvt[h][:, ch * 512:(ch + 1) * 512],
                                start=(h == 0), stop=(h == HT - 1))
                        if (kt + ch) % 2 == 0:
                            nc.vector.tensor_copy(
                                v_tiles[kt][:, ch * 512:(ch + 1) * 512], ps[:])
                        else:
                            nc.scalar.copy(
                                v_tiles[kt][:, ch * 512:(ch + 1) * 512], ps[:])

        # phase D pools (xqt DMA overlaps the tail of phase B / C2)
        xqt_pool = root.enter_context(tc.tile_pool(name="xqt", bufs=1))
        xqt = [xqt_pool.tile([P, S], F32R, name=f"xqt{a}", tag=f"xqt{a}")
               for a in range(HT)]
        for a in range(HT):
            eng = nc.sync if a < 4 else nc.gpsimd
            eng.dma_start(xqt[a][:], XqT[a * P:(a + 1) * P, :])
        e_pool = root.enter_context(tc.tile_pool(name="expT", bufs=1))
        o_pool = root.enter_context(tc.tile_pool(name="ctxo", bufs=4))
        rec_pool = root.enter_context(tc.tile_pool(name="rec", bufs=2))

        # phase D: attention in two query halves of 1024.  ps_ctx/zq are
        # created first so they land on C2's old banks (not read until
        # D2); ps_sc lands on B's old banks (no pending readers) so D1's
        # first matmul starts without a PSUM WAR wait.
        ps_ctx = root.enter_context(tc.tile_pool(name="psD2", bufs=2, space="PSUM"))
        ps_z = root.enter_context(tc.tile_pool(name="psZ", bufs=1, space="PSUM"))
        ps_sc = root.enter_context(tc.tile_pool(name="psD1", bufs=2, space="PSUM"))
        zq = ps_z.tile([P, NBLK], F32, tag="zq")

        for half in range(2):
            q0 = half * QH
            expT = [e_pool.tile([P, QH], BF16, name=f"e{half}_{jt}", tag=f"e{jt}")
                    for jt in range(KT)]
            # D1: scoreT tiles -> exp -> expT (bf16), per 512-query chunk
            for jt in range(KT):
                for qc in range(2):
                    ps = ps_sc.tile([P, 512], F32, tag="sc")
                    for a in range(HT):
                        nc.tensor.matmul(
                            ps[:], c_tiles[a][:, jt * P:(jt + 1) * P],
                            xqt[a][:, q0 + qc * 512: q0 + (qc + 1) * 512],
                            start=(a == 0), stop=(a == HT - 1))
                    nc.scalar.activation(
                        expT[jt][:, qc * 512:(qc + 1) * 512], ps[:], EXP,
                        bias=bias_cols[:, jt:jt + 1], scale=1.0)
            # D2: ctx + Z per 128-query block, accumulated over key tiles
            for blk in range(QH // P):
                bg = half * (QH // P) + blk
                ctx = ps_ctx.tile([P, H], F32, tag="ctx")
                for jt in range(KT):
                    sl = expT[jt][:, blk * P:(blk + 1) * P]
                    for ch in range(NCH):
                        nc.tensor.matmul(
                            ctx[:, ch * 512:(ch + 1) * 512], sl,
                            v_tiles[jt][:, ch * 512:(ch + 1) * 512],
                            start=(jt == 0), stop=(jt == KT - 1))
                    nc.tensor.matmul(zq[:, bg:bg + 1], sl, ones_b[:],
                                     start=(jt == 0), stop=(jt == KT - 1))
                rec = rec_pool.tile([P, 1], F32, tag="rec")
                nc.vector.reciprocal(rec[:], zq[:, bg:bg + 1])
                for ch in range(NCH):
                    ot = o_pool.tile([P, 512], F32, tag="o")
                    nc.vector.tensor_scalar_mul(
                        ot[:], ctx[:, ch * 512:(ch + 1) * 512], rec[:])
                    nc.sync.dma_start(
                        Out[bg * P:(bg + 1) * P, ch * 512:(ch + 1) * 512], ot[:])

    nc.compile()
    return nc


class _Runner:
    """Persistent PJRT executor mirroring bass2jax.run_bass_via_pjrt, built
    once so repeat kernel() calls skip jax retracing."""

    def __init__(self, nc, n_cores):
        import jax
        from jax.sharding import Mesh, PartitionSpec, NamedSharding
        from jax.experimental.shard_map import shard_map
        import concourse.mybir as mybir
        from concourse import bass2jax
        from concourse.bass2jax import _bass_exec_p, install_neuronx_cc_hook

        install_neuronx_cc_hook()
        self.jax = jax
        self.nc = nc
        self.n_cores = n_cores
        partition_name = (nc.partition_id_tensor.name
                          if nc.partition_id_tensor else None)
        in_names, out_names, out_avals = [], [], []
        for alloc in nc.m.functions[0].allocations:
            if not isinstance(alloc, mybir.MemoryLocationSet):
                continue
            name = alloc.memorylocations[0].name
            if alloc.kind == "ExternalInput":
                if name != partition_name:
                    in_names.append(name)
            elif alloc.kind == "ExternalOutput":
                out_names.append(name)
                out_avals.append(jax.core.ShapedArray(
                    tuple(alloc.tensor_shape), mybir.dt.np(alloc.dtype)))
        self.in_names, self.out_names, self.out_avals = in_names, out_names, out_avals
        n_params, n_outs = len(in_names), len(out_avals)
        self.n_params = n_params
        all_names = list(in_names) + list(out_names)
        if partition_name is not None:
            all_names.append(partition_name)

        def _body(*args):
            operands = list(args)
            if partition_name is not None:
                operands.append(bass2jax.partition_id_tensor())
            return tuple(_bass_exec_p.bind(
                *operands,
                out_avals=tuple(out_avals),
                in_names=tuple(all_names),
                out_names=tuple(out_names),
                lowering_input_output_aliases=(),
                sim_require_finite=True,
                sim_require_nnan=True,
                nc=nc,
            ))

        devices = jax.devices()[:n_cores]
        assert len(devices) == n_cores, f"need {n_cores} neuron cores"
        mesh = Mesh(np.asarray(devices), ("core",))
        in_specs = (PartitionSpec("core"),) * (n_params + n_outs)
        out_specs = (PartitionSpec("core"),) * n_outs
        donate = tuple(range(n_params, n_params + n_outs))
        self._fn = jax.jit(
            shard_map(_body, mesh=mesh, in_specs=in_specs,
                      out_specs=out_specs, check_rep=False),
            donate_argnums=donate, keep_unused=True)
        self.sharding = NamedSharding(mesh, PartitionSpec("core"))

    def run(self, in_maps):
        jax = self.jax
        in_arrs = [
            jax.device_put(
                np.concatenate([np.ascontiguousarray(m[n]) for m in in_maps], axis=0),
                self.sharding)
            for n in self.in_names
        ]
        zeros = [
            jax.device_put(
                np.zeros((self.n_cores * a.shape[0], *a.shape[1:]), a.dtype),
                self.sharding)
            for a in self.out_avals
        ]
        outs = self._fn(*in_arrs, *zeros)
        res = []
        for c in range(self.n_cores):
            res.append({
                n: np.asarray(outs[i]).reshape(self.n_cores, *self.out_avals[i].shape)[c]
                for i, n in enumerate(self.out_names)})
        return res


def _get_runner(KT, Spad2):
    key = ("runner", KT, Spad2)
    if key not in _CACHE:
        nc = _build(KT, Spad2)
        _CACHE[key] = _Runner(nc, 8)
    return _CACHE[key]


def _make_in_maps(query, key, value, Wq, Wk, Wv, mask, KT, idxs):
    Spad = KT * P
    WvT = np.ascontiguousarray(Wv.T).astype(bfloat16)
    Wq16 = Wq.astype(np.float16)
    Wk16 = Wk.astype(np.float16)
    in_maps = []
    for c in range(B):
        idx = idxs[c]
        n = len(idx)
        xqT = np.ascontiguousarray(query[c].T)
        xkT = np.zeros((H, Spad), np.float16)
        xvT = np.zeros((H, Spad), bfloat16)
        if n:
            xkT[:, :n] = key[c][idx].T.astype(np.float16)
            xvT[:, :n] = value[c][idx].T.astype(bfloat16)
        bias = np.full((Spad,), -50000.0, np.float32)
        bias[:n] = 0.0
        bias2d = np.ascontiguousarray(bias.reshape(KT, P).T)
        in_maps.append({
            "xqT": xqT, "xkT": xkT, "xvT": xvT, "wvT": WvT,
            "Wq": Wq16, "Wk": Wk16, "bias": bias2d,
        })
    return in_maps


def kernel(query, key, value, Wq, Wk, Wv, mask):
    query = np.asarray(query, dtype=np.float32)
    key = np.asarray(key, dtype=np.float32)
    value = np.asarray(value, dtype=np.float32)
    Wq = np.asarray(Wq, dtype=np.float32)
    Wk = np.asarray(Wk, dtype=np.float32)
    Wv = np.asarray(Wv, dtype=np.float32)
    mask = np.asarray(mask, dtype=np.int32)

    idxs = [np.flatnonzero(mask[c]) for c in range(B)]
    nmax = max(len(i) for i in idxs)
    KT = max(1, (nmax + P - 1) // P)
    Spad2 = min(KT * P, max(32, -(-nmax // 32) * 32))

    r = _get_runner(KT, Spad2)
    in_maps = _make_in_maps(query, key, value, Wq, Wk, Wv, mask, KT, idxs)
    res = r.run(in_maps)
    out = np.stack([res[c]["out"] for c in range(B)])

    # a batch with every key masked: reference softmax is uniform over all
    # keys (all scores equal -99999), so ctx = mean(v) for every query row
    for c in range(B):
        if len(idxs[c]) == 0:
            v_mean = (value[c].mean(0) @ Wv.T).astype(np.float32)
            out[c][:] = v_mean[None, :]
    return out
